# revision 9
# baseline (speedup 1.0000x reference)
"""Multi-head attention (B=1, L=2048, D=1024, H=16) on 8 TRN2 NeuronCores.

Sharding: tensor-parallel over heads. Core i computes heads 2i, 2i+1:
  - projections with column shards of w_q/w_k/w_v (128 cols each)
  - full attention for its 2 heads
  - partial output projection with the matching 128-row shard of w_o
Host sums the 8 partial outputs and adds the fused bias b_o + b_v @ w_o
(b_v contributes a constant row to the output; b_k cancels in softmax).

Strip-pipelined schedule (all matmuls bf16, fp32 PSUM):
  - q processed in 4 strips of 512; per (strip, kt) iteration:
      S^T pair (row-tiled K=64 matmuls, heads at PE row groups 0/64)
      -> ONE exp over [128, 1024] (both heads, single PSUM tile)
      -> AV pair (col-tiled M=64, heads at PSUM partition groups 0/64)
      -> denominator accumulate split across VectorE / GpSimd by kt parity
  - tensor queue padded with out-projection chunks of strip s-1,
    q-projection of strip s+1, vh blocks, and normalize matmuls so the
    PE never idles (keeps the 2.4 GHz p-state)
  - host supplies q strip-major and v kt-major so DMA descriptors stay
    large and vh blocks become available incrementally
  - per-strip denominator reciprocal via partition-spread DMA
"""

import os
import numpy as np
import ml_dtypes

import concourse.bass as bass
import concourse.mybir as mybir
import concourse.tile as tile
from concourse import bacc
from concourse.bass import ts
from concourse.bass_utils import run_bass_kernel_spmd

P = 128
L = 2048
D = 1024
DH = 64
NCORES = 8
NSTRIP = 4
SW = 512  # strip width (q columns per strip)
KT = D // P  # 8 contraction tiles for the projections
LT = L // P  # 16 seq tiles
BF16 = mybir.dt.bfloat16
F32 = mybir.dt.float32
AF = mybir.ActivationFunctionType
ALU = mybir.AluOpType

TRACE = False  # test.py flips this to get an NTFF profile / exec_time_ns
LAST_RESULT = {}

_CACHED_NC = None


def _build():
    nc = bacc.Bacc("TRN2", target_bir_lowering=False, debug=False, num_devices=NCORES)

    kT = nc.dram_tensor("kT", [P, KT, L], BF16, kind="ExternalInput")
    qS = nc.dram_tensor("qS", [NSTRIP, P, KT, SW], BF16, kind="ExternalInput")
    vK = nc.dram_tensor("vK", [LT, P, KT, P], BF16, kind="ExternalInput")
    wq = nc.dram_tensor("wq", [P, KT, P], BF16, kind="ExternalInput")
    wk = nc.dram_tensor("wk", [P, KT, P], BF16, kind="ExternalInput")
    wv = nc.dram_tensor("wv", [P, KT, P], BF16, kind="ExternalInput")
    bq = nc.dram_tensor("bq", [P, 1], F32, kind="ExternalInput")
    wo = nc.dram_tensor("wo", [P, D], BF16, kind="ExternalInput")
    out = nc.dram_tensor("out", [L, D], BF16, kind="ExternalOutput")

    with tile.TileContext(nc) as tc:
        with (
            tc.tile_pool(name="const", bufs=1) as const_pool,
            tc.tile_pool(name="inputs", bufs=1) as in_pool,
            tc.tile_pool(name="proj", bufs=1) as proj_pool,
            tc.tile_pool(name="work", bufs=1) as work_pool,
            tc.tile_pool(name="ps", bufs=1, space="PSUM") as psp,
            tc.tile_pool(name="ptp", bufs=2) as pt_pool,
            tc.tile_pool(name="accp", bufs=2) as acc_pool,
            tc.tile_pool(name="up", bufs=2) as u_pool,
            tc.tile_pool(name="osbp", bufs=2) as osb_pool,
        ):
            ones_c = const_pool.tile([P, P], BF16)
            nc.vector.memset(ones_c[:], 1.0)
            scr = const_pool.tile([1, 32], F32)
            nc.scalar.activation(scr[:], ones_c[0:1, 0:32], AF.Exp)

            # ---- input DMAs: weights, then k, then q strip0, v blocks,
            # then remaining q strips ----
            wq_sb = in_pool.tile([P, KT, P], BF16)
            wk_sb = in_pool.tile([P, KT, P], BF16)
            wv_sb = in_pool.tile([P, KT, P], BF16)
            bq_sb = in_pool.tile([P, 1], F32)
            wo_sb = in_pool.tile([P, D], BF16)
            nc.sync.dma_start(wk_sb[:], wk[:])
            nc.scalar.dma_start(wq_sb[:], wq[:])
            nc.gpsimd.dma_start(wv_sb[:], wv[:])
            nc.scalar.dma_start(bq_sb[:], bq[:])
            nc.gpsimd.dma_start(wo_sb[:], wo[:])

            kT_sb = in_pool.tile([P, KT, L], BF16)
            qS_sb = in_pool.tile([P, NSTRIP, KT, SW], BF16)
            vK_sb = in_pool.tile([P, LT, KT, P], BF16)
            dmae = [nc.sync, nc.scalar, nc.gpsimd]
            for c in range(6):
                t0, t1 = (c * 8) // 6, ((c + 1) * 8) // 6
                dmae[c % 3].dma_start(kT_sb[:, t0:t1, :], kT[:, t0:t1, :])
            nc.sync.dma_start(qS_sb[:, 0, :, :], qS[0])
            for b in range(LT):
                dmae[b % 3].dma_start(vK_sb[:, b, :, :], vK[b])
            for s2 in range(1, NSTRIP):
                dmae[s2 % 3].dma_start(qS_sb[:, s2, :, :], qS[s2])

            # ---- persistent SBUF tensors ----
            khT = proj_pool.tile([P, L], BF16)
            qhT = proj_pool.tile([P, L], BF16)
            vh_sb = proj_pool.tile([P, LT, P], BF16)  # [kseq, kt, dh-pair]
            lhsT_c = work_pool.tile([P, L], BF16)  # normalized concat^T
            rdb = work_pool.tile([1, NSTRIP, 2 * SW], BF16)  # 1/denominators

            def proj_k_chunk(n):
                """khT[:, n*512:(n+1)*512] (no bias: b_k cancels in softmax)."""
                ps = psp.tile([P, SW], F32, tag="mm", bufs=2, name=f"mmk_{n}")
                for t in range(KT):
                    nc.tensor.matmul(
                        ps[:],
                        wk_sb[:, t, :],
                        kT_sb[:, t, ts(n, SW)],
                        start=(t == 0),
                        stop=(t == KT - 1),
                    )
                nc.scalar.copy(khT[:, ts(n, SW)], ps[:])

            def proj_q_strip(s):
                """qhT[:, s*512:(s+1)*512] with b_q."""
                ps = psp.tile([P, SW], F32, tag="mm", bufs=2, name=f"mmq_{s}")
                for t in range(KT):
                    nc.tensor.matmul(
                        ps[:],
                        wq_sb[:, t, :],
                        qS_sb[:, s, t, :],
                        start=(t == 0),
                        stop=(t == KT - 1),
                    )
                nc.vector.tensor_scalar(
                    qhT[:, ts(s, SW)], ps[:], bq_sb[:], None, op0=ALU.add
                )

            def vh_block(b):
                """vh_sb[:, b, :] = (v @ w_v)[b-th kseq tile] directly."""
                ps = psp.tile([P, SW], F32, tag="mm", bufs=2, name=f"mmv_{b}")
                for t in range(KT):
                    nc.tensor.matmul(
                        ps[:, 0:P],
                        vK_sb[:, b, t, :],
                        wv_sb[:, t, :],
                        start=(t == 0),
                        stop=(t == KT - 1),
                    )
                nc.vector.tensor_copy(vh_sb[:, b, :], ps[:, 0:P])

            # ---- prologue ----
            for n in range(4):
                proj_k_chunk(n)
            proj_q_strip(0)
            vh_block(0)
            vh_block(1)

            # ---- strip-pipelined attention ----
            accs = [None] * NSTRIP
            us = [None] * NSTRIP
            dsps = [None] * NSTRIP
            osbs = {}

            def fin_a(s):
                """Denominator column-sums of strip s + spread DMA."""
                dps = psp.tile([P, SW], F32, tag="mm", bufs=2, name=f"dcs_{s}")
                for h in (0, 1):
                    for j, a in enumerate(accs[s]):
                        nc.tensor.matmul(
                            dps[32 * h : 32 * h + 1, :],
                            ones_c[:, 0:1],
                            a[:, ts(h, SW)],
                            start=(j == 0),
                            stop=(j == 1),
                        )
                dsb = work_pool.tile(
                    [1, 2 * SW], F32, tag="dsb", bufs=2, name=f"dsb_{s}"
                )
                nc.vector.tensor_copy(dsb[0:1, 0:SW], dps[0:1, :])
                nc.vector.tensor_copy(dsb[0:1, SW : 2 * SW], dps[32:33, :])
                dsp = work_pool.tile([P, 8], F32, tag="dsp", bufs=2, name=f"dsp_{s}")
                dsps[s] = dsp
                nc.sync.dma_start(dsp[0:DH, :], dsb[0:1, 0:SW])
                nc.gpsimd.dma_start(dsp[DH:P, :], dsb[0:1, SW : 2 * SW])

            def fin_b(s):
                """Reciprocal on the spread layout + gather back."""
                dsp = dsps[s]
                nc.vector.reciprocal(dsp[:], dsp[:])
                dspb = work_pool.tile([P, 8], BF16, tag="dspb", bufs=2, name=f"dspb_{s}")
                nc.vector.tensor_copy(dspb[:], dsp[:])
                nc.sync.dma_start(rdb[0:1, s, 0:SW], dspb[0:DH, :])
                nc.gpsimd.dma_start(rdb[0:1, s, SW : 2 * SW], dspb[DH:P, :])

            def fin_c(s):
                """Broadcast 1/d over partitions, normalize -> lhsT_c."""
                bc = psp.tile([P, SW], F32, tag="mm", bufs=2, name=f"bc_{s}")
                for h in (0, 1):
                    nc.tensor.matmul(
                        bc[ts(h, DH), :],
                        ones_c[0:1, 0:DH],
                        rdb[0:1, s, ts(h, SW)],
                    )
                nc.vector.tensor_tensor(
                    lhsT_c[:, ts(s, SW)], us[s][:], bc[:], op=ALU.mult
                )

            def outproj_chunk(s, m, n, cp_eng="v"):
                """out[s*512 + m*128, n*512] partial chunk of strip s."""
                ps = psp.tile([P, SW], F32, tag="mm", bufs=2, name=f"op_{s}_{m}_{n}")
                nc.tensor.matmul(
                    ps[:], lhsT_c[:, ts(4 * s + m, P)], wo_sb[:, ts(n, SW)]
                )
                osb = osbs.get((s, m))
                if osb is None:
                    osb = osb_pool.tile([P, D], BF16, tag="osb", name=f"osb_{s}_{m}")
                    osbs[(s, m)] = osb
                (nc.scalar.copy if cp_eng == "s" else nc.vector.tensor_copy)(
                    osb[:, ts(n, SW)], ps[:]
                )
                if n == 1:
                    (nc.sync if m % 2 == 0 else nc.gpsimd).dma_start(
                        out[ts(4 * s + m, P), :], osb[:]
                    )

            for s in range(NSTRIP):
                av = psp.tile([P, SW], F32, tag="av", bufs=2, name=f"av_{s}")
                acc_g = acc_pool.tile([P, 2 * SW], BF16, tag="accg", name=f"accg_{s}")
                acc_v = acc_pool.tile([P, 2 * SW], BF16, tag="accv", name=f"accv_{s}")
                accs[s] = (acc_g, acc_v)
                pts = [None, None]
                for kt in range(LT):
                    # scores: both heads into one PSUM tile (row groups 0/64)
                    st = psp.tile(
                        [P, 2 * SW], F32, tag="st", bufs=2, name=f"st_{s}_{kt}"
                    )
                    for h in (0, 1):
                        nc.tensor.matmul(
                            st[:, ts(h, SW)],
                            khT[ts(h, DH), ts(kt, P)],
                            qhT[ts(h, DH), ts(s, SW)],
                        )
                    # one exp for both heads; scale 1/sqrt(64) folded in
                    pt = pt_pool.tile([P, 2 * SW], BF16, tag="pt", name=f"pt_{s}_{kt}")
                    pts[kt % 2] = pt
                    nc.scalar.activation(pt[:], st[:], AF.Exp, scale=0.125)

                    # tensor pad work (keeps PE busy while exp(kt) finishes)
                    if s == 0:
                        if kt <= 13:
                            vh_block(kt + 2)
                        elif kt == 14:
                            proj_q_strip(1)
                    else:
                        if kt == 0:
                            fin_a(s - 1)
                        elif kt == 3:
                            fin_b(s - 1)
                        elif kt == 5:
                            fin_c(s - 1)
                        elif 6 <= kt <= 13:
                            outproj_chunk(s - 1, (kt - 6) // 2, (kt - 6) % 2)
                        elif kt == 14 and s + 1 < NSTRIP:
                            proj_q_strip(s + 1)

                    # AV pair of the previous kt (exp long since done)
                    def av_pair(k):
                        for h in (0, 1):
                            nc.tensor.matmul(
                                av[ts(h, DH), :],
                                vh_sb[:, k, ts(h, DH)],
                                pts[k % 2][:, ts(h, SW)],
                                start=(k == 0),
                                stop=(k == LT - 1),
                            )

                    if kt > 0:
                        av_pair(kt - 1)
                    # two independent denominator chains (gpsimd slower/op)
                    if kt % 3 == 0:
                        if kt == 0:
                            nc.gpsimd.tensor_copy(acc_g[:], pt[:])
                        else:
                            nc.gpsimd.tensor_tensor(
                                acc_g[:], acc_g[:], pt[:], op=ALU.add
                            )
                    else:
                        if kt == 1:
                            nc.vector.tensor_copy(acc_v[:], pt[:])
                        else:
                            nc.vector.tensor_tensor(
                                acc_v[:], acc_v[:], pt[:], op=ALU.add
                            )
                av_pair(LT - 1)
                # unnormalized attention out of this strip -> SBUF
                u = u_pool.tile([P, SW], F32, tag="u", name=f"u_{s}")
                us[s] = u
                nc.vector.tensor_copy(u[:], av[:])

            # ---- epilogue for the last strip ----
            s = NSTRIP - 1
            fin_a(s)
            fin_b(s)
            fin_c(s)
            for m in range(4):
                for n in range(2):
                    outproj_chunk(s, m, n, cp_eng="s" if (2 * m + n) % 2 else "v")

    nc.compile()
    return nc


def kernel(q, k, v, w_q, b_q, w_k, b_k, w_v, b_v, w_o, b_o):
    global _CACHED_NC, LAST_RESULT
    if _CACHED_NC is None:
        _CACHED_NC = _build()
    nc = _CACHED_NC

    bf16 = ml_dtypes.bfloat16

    def tile_T(x):  # [L, D] -> [128, D//128, L] contiguous
        xt = np.asarray(x, np.float32)[0].T  # [D, L]
        return np.ascontiguousarray(
            xt.reshape(D // P, P, L).transpose(1, 0, 2)
        ).astype(bf16)

    def tile_w(w):  # [D, 128] -> [128, D//128, 128] contiguous
        return np.ascontiguousarray(
            w.reshape(D // P, P, P).transpose(1, 0, 2)
        ).astype(bf16)

    k2 = tile_T(k)
    # q strip-major: [NSTRIP, 128, KT, 512]
    q2 = np.ascontiguousarray(
        tile_T(q).reshape(P, KT, NSTRIP, SW).transpose(2, 0, 1, 3)
    )
    # v kt-major: [LT, 128, KT, 128]
    v2 = np.ascontiguousarray(
        tile_T(v).reshape(P, KT, LT, P).transpose(2, 0, 1, 3)
    )
    w_q = np.asarray(w_q, np.float32)
    w_k = np.asarray(w_k, np.float32)
    w_v = np.asarray(w_v, np.float32)
    w_o = np.asarray(w_o, np.float32)
    b_q = np.asarray(b_q, np.float32)
    b_v = np.asarray(b_v, np.float32)
    b_o = np.asarray(b_o, np.float32)

    in_maps = []
    for i in range(NCORES):
        sl = slice(P * i, P * (i + 1))
        in_maps.append(
            {
                "kT": k2,
                "qS": q2,
                "vK": v2,
                "wq": tile_w(w_q[:, sl]),
                "wk": tile_w(w_k[:, sl]),
                "wv": tile_w(w_v[:, sl]),
                "bq": np.ascontiguousarray(b_q[sl]).reshape(P, 1),
                "wo": np.ascontiguousarray(w_o[sl, :]).astype(bf16),
            }
        )

    kwargs = {}
    if TRACE:
        import shutil

        tdir = "/tmp/bass_trace"
        shutil.rmtree(tdir, ignore_errors=True)
        os.makedirs(tdir, exist_ok=True)
        kwargs["tmpdir"] = tdir
    res = run_bass_kernel_spmd(nc, in_maps, list(range(NCORES)), trace=TRACE, **kwargs)
    LAST_RESULT = {
        "exec_time_ns": res.exec_time_ns,
        "trace_path": (res.instructions_and_trace or (None, None))[1],
    }
    acc = np.zeros((L, D), np.float64)
    for i in range(NCORES):
        acc += res.results[i]["out"].astype(np.float64)
    # b_k cancels in softmax; b_v and b_o contribute a constant output row
    acc += (b_o + b_v @ w_o).astype(np.float64)
    return acc.astype(np.float32).reshape(1, L, D)


# revision 10
# speedup vs baseline: 1.0604x; 1.0604x over previous
"""Multi-head attention (B=1, L=2048, D=1024, H=16) on 8 TRN2 NeuronCores.

Sharding: tensor-parallel over heads. Core i computes heads 2i, 2i+1:
  - projections with column shards of w_q/w_k/w_v (128 cols each)
  - full attention for its 2 heads
  - partial output projection with the matching 128-row shard of w_o
Host sums the 8 partial outputs and adds the fused bias b_o + b_v @ w_o
(b_v contributes a constant row to the output; b_k cancels in softmax).

Strip-pipelined schedule (all matmuls bf16, fp32 PSUM):
  - q processed in 4 strips of 512; per (strip, kt) iteration:
      S^T pair (row-tiled K=64 matmuls, heads at PE row groups 0/64)
      -> ONE exp over [128, 1024] (both heads, single PSUM tile)
      -> AV pair (col-tiled M=64, heads at PSUM partition groups 0/64)
      -> denominator accumulate split across VectorE / GpSimd by kt parity
  - tensor queue padded with out-projection chunks of strip s-1,
    q-projection of strip s+1, vh blocks, and normalize matmuls so the
    PE never idles (keeps the 2.4 GHz p-state)
  - host supplies q strip-major and v kt-major so DMA descriptors stay
    large and vh blocks become available incrementally
  - per-strip denominator reciprocal via partition-spread DMA
"""

import os
import numpy as np
import ml_dtypes

import concourse.bass as bass
import concourse.mybir as mybir
import concourse.tile as tile
from concourse import bacc
from concourse.bass import ts
from concourse.bass_utils import run_bass_kernel_spmd

P = 128
L = 2048
D = 1024
DH = 64
NCORES = 8
NSTRIP = 4
SW = 512  # strip width (q columns per strip)
KT = D // P  # 8 contraction tiles for the projections
LT = L // P  # 16 seq tiles
BF16 = mybir.dt.bfloat16
F32 = mybir.dt.float32
AF = mybir.ActivationFunctionType
ALU = mybir.AluOpType

TRACE = False  # test.py flips this to get an NTFF profile / exec_time_ns
LAST_RESULT = {}

_CACHED_NC = None


def _build():
    nc = bacc.Bacc("TRN2", target_bir_lowering=False, debug=False, num_devices=NCORES)

    kT = nc.dram_tensor("kT", [P, KT, L], BF16, kind="ExternalInput")
    qS = nc.dram_tensor("qS", [NSTRIP, P, KT, SW], BF16, kind="ExternalInput")
    vK = nc.dram_tensor("vK", [LT, P, KT, P], BF16, kind="ExternalInput")
    wq = nc.dram_tensor("wq", [P, KT, P], BF16, kind="ExternalInput")
    wk = nc.dram_tensor("wk", [P, KT, P], BF16, kind="ExternalInput")
    wv = nc.dram_tensor("wv", [P, KT, P], BF16, kind="ExternalInput")
    bq = nc.dram_tensor("bq", [P, 1], F32, kind="ExternalInput")
    wo = nc.dram_tensor("wo", [P, D], BF16, kind="ExternalInput")
    out = nc.dram_tensor("out", [L, D], BF16, kind="ExternalOutput")

    with tile.TileContext(nc) as tc:
        with (
            tc.tile_pool(name="const", bufs=1) as const_pool,
            tc.tile_pool(name="inputs", bufs=1) as in_pool,
            tc.tile_pool(name="proj", bufs=1) as proj_pool,
            tc.tile_pool(name="work", bufs=1) as work_pool,
            tc.tile_pool(name="ps", bufs=1, space="PSUM") as psp,
            tc.tile_pool(name="ptp", bufs=3) as pt_pool,
            tc.tile_pool(name="accp", bufs=2) as acc_pool,
            tc.tile_pool(name="up", bufs=2) as u_pool,
            tc.tile_pool(name="osbp", bufs=2) as osb_pool,
        ):
            ones_c = const_pool.tile([P, P], BF16)
            nc.vector.memset(ones_c[:], 1.0)
            scr = const_pool.tile([1, 32], F32)
            nc.scalar.activation(scr[:], ones_c[0:1, 0:32], AF.Exp)

            # ---- input DMAs: weights, then k, then q strip0, v blocks,
            # then remaining q strips ----
            wq_sb = in_pool.tile([P, KT, P], BF16)
            wk_sb = in_pool.tile([P, KT, P], BF16)
            wv_sb = in_pool.tile([P, KT, P], BF16)
            bq_sb = in_pool.tile([P, 1], F32)
            wo_sb = in_pool.tile([P, D], BF16)
            nc.sync.dma_start(wk_sb[:], wk[:])
            nc.scalar.dma_start(wq_sb[:], wq[:])
            nc.gpsimd.dma_start(wv_sb[:], wv[:])
            nc.scalar.dma_start(bq_sb[:], bq[:])
            nc.gpsimd.dma_start(wo_sb[:], wo[:])

            kT_sb = in_pool.tile([P, KT, L], BF16)
            qS_sb = in_pool.tile([P, NSTRIP, KT, SW], BF16)
            vK_sb = in_pool.tile([P, LT, KT, P], BF16)
            dmae = [nc.sync, nc.scalar, nc.gpsimd]
            for c in range(6):
                t0, t1 = (c * 8) // 6, ((c + 1) * 8) // 6
                dmae[c % 3].dma_start(kT_sb[:, t0:t1, :], kT[:, t0:t1, :])
            nc.sync.dma_start(qS_sb[:, 0, :, :], qS[0])
            for b in range(LT):
                dmae[b % 3].dma_start(vK_sb[:, b, :, :], vK[b])
            for s2 in range(1, NSTRIP):
                dmae[s2 % 3].dma_start(qS_sb[:, s2, :, :], qS[s2])

            # ---- persistent SBUF tensors ----
            khT = proj_pool.tile([P, L], BF16)
            qhT = proj_pool.tile([P, L], BF16)
            vh_sb = proj_pool.tile([P, LT, P], BF16)  # [kseq, kt, dh-pair]
            lhsT_c = work_pool.tile([P, L], BF16)  # normalized concat^T
            rdb = work_pool.tile([1, NSTRIP, 2 * SW], BF16)  # 1/denominators

            def proj_k_chunk(n):
                """khT[:, n*512:(n+1)*512] (no bias: b_k cancels in softmax)."""
                ps = psp.tile([P, SW], F32, tag="mm", bufs=2, name=f"mmk_{n}")
                for t in range(KT):
                    nc.tensor.matmul(
                        ps[:],
                        wk_sb[:, t, :],
                        kT_sb[:, t, ts(n, SW)],
                        start=(t == 0),
                        stop=(t == KT - 1),
                    )
                nc.scalar.copy(khT[:, ts(n, SW)], ps[:])

            def proj_q_strip(s):
                """qhT[:, s*512:(s+1)*512] with b_q."""
                ps = psp.tile([P, SW], F32, tag="mm", bufs=2, name=f"mmq_{s}")
                for t in range(KT):
                    nc.tensor.matmul(
                        ps[:],
                        wq_sb[:, t, :],
                        qS_sb[:, s, t, :],
                        start=(t == 0),
                        stop=(t == KT - 1),
                    )
                nc.vector.tensor_scalar(
                    qhT[:, ts(s, SW)], ps[:], bq_sb[:], None, op0=ALU.add
                )

            def vh_block(b):
                """vh_sb[:, b, :] = (v @ w_v)[b-th kseq tile] directly."""
                ps = psp.tile([P, SW], F32, tag="mm", bufs=2, name=f"mmv_{b}")
                for t in range(KT):
                    nc.tensor.matmul(
                        ps[:, 0:P],
                        vK_sb[:, b, t, :],
                        wv_sb[:, t, :],
                        start=(t == 0),
                        stop=(t == KT - 1),
                    )
                nc.vector.tensor_copy(vh_sb[:, b, :], ps[:, 0:P])

            # ---- prologue ----
            for n in range(4):
                proj_k_chunk(n)
            proj_q_strip(0)
            vh_block(0)
            vh_block(1)

            # ---- strip-pipelined attention ----
            accs = [None] * NSTRIP
            us = [None] * NSTRIP
            dsps = [None] * NSTRIP
            osbs = {}

            def fin_a(s):
                """Denominator column-sums of strip s + spread DMA."""
                dps = psp.tile([P, SW], F32, tag="mm", bufs=2, name=f"dcs_{s}")
                for h in (0, 1):
                    for j, a in enumerate(accs[s]):
                        nc.tensor.matmul(
                            dps[32 * h : 32 * h + 1, :],
                            ones_c[:, 0:1],
                            a[:, ts(h, SW)],
                            start=(j == 0),
                            stop=(j == 1),
                        )
                dsb = work_pool.tile(
                    [1, 2 * SW], F32, tag="dsb", bufs=2, name=f"dsb_{s}"
                )
                nc.scalar.copy(dsb[0:1, 0:SW], dps[0:1, :])
                nc.scalar.copy(dsb[0:1, SW : 2 * SW], dps[32:33, :])
                dsp = work_pool.tile([P, 8], F32, tag="dsp", bufs=2, name=f"dsp_{s}")
                dsps[s] = dsp
                nc.sync.dma_start(dsp[0:DH, :], dsb[0:1, 0:SW])
                nc.gpsimd.dma_start(dsp[DH:P, :], dsb[0:1, SW : 2 * SW])

            def fin_b(s):
                """Reciprocal on the spread layout + gather back."""
                dsp = dsps[s]
                nc.vector.reciprocal(dsp[:], dsp[:])
                dspb = work_pool.tile([P, 8], BF16, tag="dspb", bufs=2, name=f"dspb_{s}")
                nc.vector.tensor_copy(dspb[:], dsp[:])
                nc.sync.dma_start(rdb[0:1, s, 0:SW], dspb[0:DH, :])
                nc.gpsimd.dma_start(rdb[0:1, s, SW : 2 * SW], dspb[DH:P, :])

            def fin_c(s):
                """Broadcast 1/d over partitions, normalize -> lhsT_c."""
                bc = psp.tile([P, SW], F32, tag="mm", bufs=2, name=f"bc_{s}")
                for h in (0, 1):
                    nc.tensor.matmul(
                        bc[ts(h, DH), :],
                        ones_c[0:1, 0:DH],
                        rdb[0:1, s, ts(h, SW)],
                    )
                nc.vector.tensor_tensor(
                    lhsT_c[:, ts(s, SW)], us[s][:], bc[:], op=ALU.mult
                )

            def outproj_chunk(s, m, n, cp_eng="v"):
                """out[s*512 + m*128, n*512] partial chunk of strip s."""
                ps = psp.tile([P, SW], F32, tag="mm", bufs=2, name=f"op_{s}_{m}_{n}")
                nc.tensor.matmul(
                    ps[:], lhsT_c[:, ts(4 * s + m, P)], wo_sb[:, ts(n, SW)]
                )
                osb = osbs.get((s, m))
                if osb is None:
                    osb = osb_pool.tile([P, D], BF16, tag="osb", name=f"osb_{s}_{m}")
                    osbs[(s, m)] = osb
                (nc.scalar.copy if cp_eng == "s" else nc.vector.tensor_copy)(
                    osb[:, ts(n, SW)], ps[:]
                )
                if n == 1:
                    (nc.sync if m % 2 == 0 else nc.gpsimd).dma_start(
                        out[ts(4 * s + m, P), :], osb[:]
                    )

            for s in range(NSTRIP):
                av = psp.tile([P, SW], F32, tag="av", bufs=2, name=f"av_{s}")
                acc_g = acc_pool.tile([P, 2 * SW], BF16, tag="accg", name=f"accg_{s}")
                acc_v = acc_pool.tile([P, 2 * SW], BF16, tag="accv", name=f"accv_{s}")
                accs[s] = (acc_g, acc_v)
                pts = [None, None, None]
                for kt in range(LT):
                    # scores: both heads into one PSUM tile (row groups 0/64)
                    st = psp.tile(
                        [P, 2 * SW], F32, tag="st", bufs=2, name=f"st_{s}_{kt}"
                    )
                    for h in (0, 1):
                        nc.tensor.matmul(
                            st[:, ts(h, SW)],
                            khT[ts(h, DH), ts(kt, P)],
                            qhT[ts(h, DH), ts(s, SW)],
                        )
                    # one exp for both heads; scale 1/sqrt(64) folded in
                    pt = pt_pool.tile([P, 2 * SW], BF16, tag="pt", name=f"pt_{s}_{kt}")
                    pts[kt % 3] = pt
                    nc.scalar.activation(pt[:], st[:], AF.Exp, scale=0.125)

                    # tensor pad work (keeps PE busy while exp(kt) finishes)
                    if s == 0:
                        if kt <= 13:
                            vh_block(kt + 2)
                        elif kt == 14:
                            proj_q_strip(1)
                    else:
                        if kt == 0:
                            fin_a(s - 1)
                        elif kt == 3:
                            fin_b(s - 1)
                        elif kt == 5:
                            fin_c(s - 1)
                        elif 6 <= kt <= 13:
                            outproj_chunk(s - 1, (kt - 6) // 2, (kt - 6) % 2)
                        elif kt == 14 and s + 1 < NSTRIP:
                            proj_q_strip(s + 1)

                    # AV pair of the previous kt (exp long since done)
                    def av_pair(k):
                        for h in (0, 1):
                            nc.tensor.matmul(
                                av[ts(h, DH), :],
                                vh_sb[:, k, ts(h, DH)],
                                pts[k % 3][:, ts(h, SW)],
                                start=(k == 0),
                                stop=(k == LT - 1),
                            )

                    if kt > 0:
                        av_pair(kt - 1)
                    # two independent denominator chains; gpsimd (slow per
                    # op) only gets mid-strip slots so it never lags fin_a
                    if kt in (2, 5, 8, 11):
                        if kt == 2:
                            nc.gpsimd.tensor_copy(acc_g[:], pt[:])
                        else:
                            nc.gpsimd.tensor_tensor(
                                acc_g[:], acc_g[:], pt[:], op=ALU.add
                            )
                    else:
                        if kt == 0:
                            nc.vector.tensor_copy(acc_v[:], pt[:])
                        else:
                            nc.vector.tensor_tensor(
                                acc_v[:], acc_v[:], pt[:], op=ALU.add
                            )
                av_pair(LT - 1)
                # unnormalized attention out of this strip -> SBUF
                u = u_pool.tile([P, SW], F32, tag="u", name=f"u_{s}")
                us[s] = u
                nc.scalar.copy(u[:], av[:])

            # ---- epilogue for the last strip ----
            s = NSTRIP - 1
            fin_a(s)
            fin_b(s)
            fin_c(s)
            for m in range(4):
                for n in range(2):
                    outproj_chunk(s, m, n, cp_eng="s" if (2 * m + n) % 2 else "v")

    nc.compile()
    return nc


def kernel(q, k, v, w_q, b_q, w_k, b_k, w_v, b_v, w_o, b_o):
    global _CACHED_NC, LAST_RESULT
    if _CACHED_NC is None:
        _CACHED_NC = _build()
    nc = _CACHED_NC

    bf16 = ml_dtypes.bfloat16

    def tile_T(x):  # [L, D] -> [128, D//128, L] contiguous
        xt = np.asarray(x, np.float32)[0].T  # [D, L]
        return np.ascontiguousarray(
            xt.reshape(D // P, P, L).transpose(1, 0, 2)
        ).astype(bf16)

    def tile_w(w):  # [D, 128] -> [128, D//128, 128] contiguous
        return np.ascontiguousarray(
            w.reshape(D // P, P, P).transpose(1, 0, 2)
        ).astype(bf16)

    k2 = tile_T(k)
    # q strip-major: [NSTRIP, 128, KT, 512]
    q2 = np.ascontiguousarray(
        tile_T(q).reshape(P, KT, NSTRIP, SW).transpose(2, 0, 1, 3)
    )
    # v kt-major: [LT, 128, KT, 128]
    v2 = np.ascontiguousarray(
        tile_T(v).reshape(P, KT, LT, P).transpose(2, 0, 1, 3)
    )
    w_q = np.asarray(w_q, np.float32)
    w_k = np.asarray(w_k, np.float32)
    w_v = np.asarray(w_v, np.float32)
    w_o = np.asarray(w_o, np.float32)
    b_q = np.asarray(b_q, np.float32)
    b_v = np.asarray(b_v, np.float32)
    b_o = np.asarray(b_o, np.float32)

    in_maps = []
    for i in range(NCORES):
        sl = slice(P * i, P * (i + 1))
        in_maps.append(
            {
                "kT": k2,
                "qS": q2,
                "vK": v2,
                "wq": tile_w(w_q[:, sl]),
                "wk": tile_w(w_k[:, sl]),
                "wv": tile_w(w_v[:, sl]),
                "bq": np.ascontiguousarray(b_q[sl]).reshape(P, 1),
                "wo": np.ascontiguousarray(w_o[sl, :]).astype(bf16),
            }
        )

    kwargs = {}
    if TRACE:
        import shutil

        tdir = "/tmp/bass_trace"
        shutil.rmtree(tdir, ignore_errors=True)
        os.makedirs(tdir, exist_ok=True)
        kwargs["tmpdir"] = tdir
    res = run_bass_kernel_spmd(nc, in_maps, list(range(NCORES)), trace=TRACE, **kwargs)
    LAST_RESULT = {
        "exec_time_ns": res.exec_time_ns,
        "trace_path": (res.instructions_and_trace or (None, None))[1],
    }
    acc = np.zeros((L, D), np.float64)
    for i in range(NCORES):
        acc += res.results[i]["out"].astype(np.float64)
    # b_k cancels in softmax; b_v and b_o contribute a constant output row
    acc += (b_o + b_v @ w_o).astype(np.float64)
    return acc.astype(np.float32).reshape(1, L, D)


# revision 11
# speedup vs baseline: 1.0638x; 1.0033x over previous
"""Multi-head attention (B=1, L=2048, D=1024, H=16) on 8 TRN2 NeuronCores.

Sharding: tensor-parallel over heads. Core i computes heads 2i, 2i+1:
  - projections with column shards of w_q/w_k/w_v (128 cols each)
  - full attention for its 2 heads
  - partial output projection with the matching 128-row shard of w_o
Host sums the 8 partial outputs and adds the fused bias b_o + b_v @ w_o
(b_v contributes a constant row to the output; b_k cancels in softmax).

Strip-pipelined schedule (all matmuls bf16, fp32 PSUM):
  - q processed in 4 strips of 512; per (strip, kt) iteration:
      S^T pair (row-tiled K=64 matmuls, heads at PE row groups 0/64)
      -> ONE exp over [128, 1024] (both heads, single PSUM tile)
      -> AV pair (col-tiled M=64, heads at PSUM partition groups 0/64)
      -> denominator accumulate split across VectorE / GpSimd by kt parity
  - tensor queue padded with out-projection chunks of strip s-1,
    q-projection of strip s+1, vh blocks, and normalize matmuls so the
    PE never idles (keeps the 2.4 GHz p-state)
  - host supplies q strip-major and v kt-major so DMA descriptors stay
    large and vh blocks become available incrementally
  - per-strip denominator reciprocal via partition-spread DMA
"""

import os
import numpy as np
import ml_dtypes

import concourse.bass as bass
import concourse.mybir as mybir
import concourse.tile as tile
from concourse import bacc
from concourse.bass import ts
from concourse.bass_utils import run_bass_kernel_spmd

P = 128
L = 2048
D = 1024
DH = 64
NCORES = 8
NSTRIP = 4
SW = 512  # strip width (q columns per strip)
KT = D // P  # 8 contraction tiles for the projections
LT = L // P  # 16 seq tiles
BF16 = mybir.dt.bfloat16
F32 = mybir.dt.float32
AF = mybir.ActivationFunctionType
ALU = mybir.AluOpType

TRACE = False  # test.py flips this to get an NTFF profile / exec_time_ns
LAST_RESULT = {}

_CACHED_NC = None


def _build():
    nc = bacc.Bacc("TRN2", target_bir_lowering=False, debug=False, num_devices=NCORES)

    kT = nc.dram_tensor("kT", [P, KT, L], BF16, kind="ExternalInput")
    qS = nc.dram_tensor("qS", [NSTRIP, P, KT, SW], BF16, kind="ExternalInput")
    vK = nc.dram_tensor("vK", [LT, P, KT, P], BF16, kind="ExternalInput")
    wq = nc.dram_tensor("wq", [P, KT, P], BF16, kind="ExternalInput")
    wk = nc.dram_tensor("wk", [P, KT, P], BF16, kind="ExternalInput")
    wv = nc.dram_tensor("wv", [P, KT, P], BF16, kind="ExternalInput")
    bq = nc.dram_tensor("bq", [P, 1], F32, kind="ExternalInput")
    wo = nc.dram_tensor("wo", [P, D], BF16, kind="ExternalInput")
    out = nc.dram_tensor("out", [L, D], BF16, kind="ExternalOutput")

    with tile.TileContext(nc) as tc:
        with (
            tc.tile_pool(name="const", bufs=1) as const_pool,
            tc.tile_pool(name="inputs", bufs=1) as in_pool,
            tc.tile_pool(name="proj", bufs=1) as proj_pool,
            tc.tile_pool(name="work", bufs=1) as work_pool,
            tc.tile_pool(name="ps", bufs=1, space="PSUM") as psp,
            tc.tile_pool(name="ptp", bufs=3) as pt_pool,
            tc.tile_pool(name="accp", bufs=2) as acc_pool,
            tc.tile_pool(name="up", bufs=2) as u_pool,
            tc.tile_pool(name="osbp", bufs=2) as osb_pool,
        ):
            ones_c = const_pool.tile([P, P], BF16)
            nc.vector.memset(ones_c[:], 1.0)
            scr = const_pool.tile([1, 32], F32)
            nc.scalar.activation(scr[:], ones_c[0:1, 0:32], AF.Exp)

            # ---- input DMAs: weights, then k, then q strip0, v blocks,
            # then remaining q strips ----
            wq_sb = in_pool.tile([P, KT, P], BF16)
            wk_sb = in_pool.tile([P, KT, P], BF16)
            wv_sb = in_pool.tile([P, KT, P], BF16)
            bq_sb = in_pool.tile([P, 1], F32)
            wo_sb = in_pool.tile([P, D], BF16)
            nc.sync.dma_start(wk_sb[:], wk[:])
            nc.scalar.dma_start(wq_sb[:], wq[:])
            nc.gpsimd.dma_start(wv_sb[:], wv[:])
            nc.scalar.dma_start(bq_sb[:], bq[:])
            nc.gpsimd.dma_start(wo_sb[:], wo[:])

            kT_sb = in_pool.tile([P, KT, L], BF16)
            qS_sb = in_pool.tile([P, NSTRIP, KT, SW], BF16)
            vK_sb = in_pool.tile([P, LT, KT, P], BF16)
            dmae = [nc.sync, nc.scalar, nc.gpsimd]
            for c in range(6):
                t0, t1 = (c * 8) // 6, ((c + 1) * 8) // 6
                dmae[c % 3].dma_start(kT_sb[:, t0:t1, :], kT[:, t0:t1, :])
            nc.sync.dma_start(qS_sb[:, 0, :, :], qS[0])
            for b in range(LT):
                dmae[b % 3].dma_start(vK_sb[:, b, :, :], vK[b])
            for s2 in range(1, NSTRIP):
                dmae[s2 % 3].dma_start(qS_sb[:, s2, :, :], qS[s2])

            # ---- persistent SBUF tensors ----
            khT = proj_pool.tile([P, L], BF16)
            qhT = proj_pool.tile([P, L], BF16)
            vh_sb = proj_pool.tile([P, LT, P], BF16)  # [kseq, kt, dh-pair]
            lhsT_c = work_pool.tile([P, L], BF16)  # normalized concat^T
            rdb = work_pool.tile([1, NSTRIP, 2 * SW], BF16)  # 1/denominators

            def proj_k_chunk(n):
                """khT[:, n*512:(n+1)*512] (no bias: b_k cancels in softmax)."""
                ps = psp.tile([P, SW], F32, tag="mm", bufs=2, name=f"mmk_{n}")
                for t in range(KT):
                    nc.tensor.matmul(
                        ps[:],
                        wk_sb[:, t, :],
                        kT_sb[:, t, ts(n, SW)],
                        start=(t == 0),
                        stop=(t == KT - 1),
                    )
                nc.scalar.copy(khT[:, ts(n, SW)], ps[:])

            _qps = {}

            def proj_q_part(s, half):
                """Half of the q-projection for strip s (pad-slot sized)."""
                if half == 0:
                    _qps[s] = psp.tile(
                        [P, SW], F32, tag="mm", bufs=2, name=f"mmq_{s}"
                    )
                ps = _qps[s]
                for t in range(4 * half, 4 * half + 4):
                    nc.tensor.matmul(
                        ps[:],
                        wq_sb[:, t, :],
                        qS_sb[:, s, t, :],
                        start=(t == 0),
                        stop=(t == KT - 1),
                    )
                if half == 1:
                    nc.vector.tensor_scalar(
                        qhT[:, ts(s, SW)], ps[:], bq_sb[:], None, op0=ALU.add
                    )

            def proj_q_strip(s):
                proj_q_part(s, 0)
                proj_q_part(s, 1)

            def vh_block(b):
                """vh_sb[:, b, :] = (v @ w_v)[b-th kseq tile] directly."""
                ps = psp.tile([P, SW], F32, tag="mm", bufs=2, name=f"mmv_{b}")
                for t in range(KT):
                    nc.tensor.matmul(
                        ps[:, 0:P],
                        vK_sb[:, b, t, :],
                        wv_sb[:, t, :],
                        start=(t == 0),
                        stop=(t == KT - 1),
                    )
                nc.vector.tensor_copy(vh_sb[:, b, :], ps[:, 0:P])

            # ---- prologue ----
            for n in range(4):
                proj_k_chunk(n)
            proj_q_strip(0)
            vh_block(0)
            vh_block(1)

            # ---- strip-pipelined attention ----
            accs = [None] * NSTRIP
            us = [None] * NSTRIP
            dsps = [None] * NSTRIP
            osbs = {}

            def fin_a(s):
                """Denominator column-sums of strip s + spread DMA."""
                acc_g, acc_v = accs[s]
                nc.vector.tensor_tensor(acc_v[:], acc_v[:], acc_g[:], op=ALU.add)
                dps = psp.tile([P, SW], F32, tag="mm", bufs=2, name=f"dcs_{s}")
                for h in (0, 1):
                    nc.tensor.matmul(
                        dps[32 * h : 32 * h + 1, :],
                        ones_c[:, 0:1],
                        acc_v[:, ts(h, SW)],
                    )
                dsb = work_pool.tile(
                    [1, 2 * SW], F32, tag="dsb", bufs=2, name=f"dsb_{s}"
                )
                nc.scalar.copy(dsb[0:1, 0:SW], dps[0:1, :])
                nc.scalar.copy(dsb[0:1, SW : 2 * SW], dps[32:33, :])
                dsp = work_pool.tile([P, 8], F32, tag="dsp", bufs=2, name=f"dsp_{s}")
                dsps[s] = dsp
                nc.sync.dma_start(dsp[0:DH, :], dsb[0:1, 0:SW])
                nc.gpsimd.dma_start(dsp[DH:P, :], dsb[0:1, SW : 2 * SW])

            def fin_b(s):
                """Reciprocal on the spread layout + gather back."""
                dsp = dsps[s]
                nc.vector.reciprocal(dsp[:], dsp[:])
                dspb = work_pool.tile([P, 8], BF16, tag="dspb", bufs=2, name=f"dspb_{s}")
                nc.vector.tensor_copy(dspb[:], dsp[:])
                nc.sync.dma_start(rdb[0:1, s, 0:SW], dspb[0:DH, :])
                nc.gpsimd.dma_start(rdb[0:1, s, SW : 2 * SW], dspb[DH:P, :])

            def fin_c(s):
                """Broadcast 1/d over partitions, normalize -> lhsT_c."""
                bc = psp.tile([P, SW], F32, tag="mm", bufs=2, name=f"bc_{s}")
                for h in (0, 1):
                    nc.tensor.matmul(
                        bc[ts(h, DH), :],
                        ones_c[0:1, 0:DH],
                        rdb[0:1, s, ts(h, SW)],
                    )
                nc.vector.tensor_tensor(
                    lhsT_c[:, ts(s, SW)], us[s][:], bc[:], op=ALU.mult
                )

            def outproj_chunk(s, m, n, cp_eng="v"):
                """out[s*512 + m*128, n*512] partial chunk of strip s."""
                ps = psp.tile([P, SW], F32, tag="mm", bufs=2, name=f"op_{s}_{m}_{n}")
                nc.tensor.matmul(
                    ps[:], lhsT_c[:, ts(4 * s + m, P)], wo_sb[:, ts(n, SW)]
                )
                osb = osbs.get((s, m))
                if osb is None:
                    osb = osb_pool.tile([P, D], BF16, tag="osb", name=f"osb_{s}_{m}")
                    osbs[(s, m)] = osb
                (nc.scalar.copy if cp_eng == "s" else nc.vector.tensor_copy)(
                    osb[:, ts(n, SW)], ps[:]
                )
                if n == 1:
                    (nc.sync if m % 2 == 0 else nc.gpsimd).dma_start(
                        out[ts(4 * s + m, P), :], osb[:]
                    )

            for s in range(NSTRIP):
                av = psp.tile([P, SW], F32, tag="av", bufs=2, name=f"av_{s}")
                acc_g = acc_pool.tile([P, 2 * SW], BF16, tag="accg", name=f"accg_{s}")
                acc_v = acc_pool.tile([P, 2 * SW], BF16, tag="accv", name=f"accv_{s}")
                accs[s] = (acc_g, acc_v)
                pts = [None, None, None]
                for kt in range(LT):
                    # scores: both heads into one PSUM tile (row groups 0/64)
                    st = psp.tile(
                        [P, 2 * SW], F32, tag="st", bufs=2, name=f"st_{s}_{kt}"
                    )
                    for h in (0, 1):
                        nc.tensor.matmul(
                            st[:, ts(h, SW)],
                            khT[ts(h, DH), ts(kt, P)],
                            qhT[ts(h, DH), ts(s, SW)],
                        )
                    # one exp for both heads; scale 1/sqrt(64) folded in
                    pt = pt_pool.tile([P, 2 * SW], BF16, tag="pt", name=f"pt_{s}_{kt}")
                    pts[kt % 3] = pt
                    nc.scalar.activation(pt[:], st[:], AF.Exp, scale=0.125)

                    # tensor pad work (keeps PE busy while exp(kt) finishes)
                    if s == 0:
                        if kt <= 13:
                            vh_block(kt + 2)
                        elif kt == 14:
                            proj_q_part(1, 0)
                        elif kt == 15:
                            proj_q_part(1, 1)
                    else:
                        if kt == 0:
                            fin_a(s - 1)
                        elif kt == 3:
                            fin_b(s - 1)
                        elif kt == 5:
                            fin_c(s - 1)
                        elif 6 <= kt <= 13:
                            outproj_chunk(s - 1, (kt - 6) // 2, (kt - 6) % 2)
                        elif kt == 14 and s + 1 < NSTRIP:
                            proj_q_part(s + 1, 0)
                        elif kt == 15 and s + 1 < NSTRIP:
                            proj_q_part(s + 1, 1)

                    # AV pair of the previous kt (exp long since done)
                    def av_pair(k):
                        for h in (0, 1):
                            nc.tensor.matmul(
                                av[ts(h, DH), :],
                                vh_sb[:, k, ts(h, DH)],
                                pts[k % 3][:, ts(h, SW)],
                                start=(k == 0),
                                stop=(k == LT - 1),
                            )

                    if kt > 0:
                        av_pair(kt - 1)
                    # two independent denominator chains; gpsimd (slow per
                    # op) only gets mid-strip slots so it never lags fin_a
                    if kt in (2, 5, 8, 11):
                        if kt == 2:
                            nc.gpsimd.tensor_copy(acc_g[:], pt[:])
                        else:
                            nc.gpsimd.tensor_tensor(
                                acc_g[:], acc_g[:], pt[:], op=ALU.add
                            )
                    else:
                        if kt == 0:
                            nc.vector.tensor_copy(acc_v[:], pt[:])
                        else:
                            nc.vector.tensor_tensor(
                                acc_v[:], acc_v[:], pt[:], op=ALU.add
                            )
                av_pair(LT - 1)
                # unnormalized attention out of this strip -> SBUF
                u = u_pool.tile([P, SW], F32, tag="u", name=f"u_{s}")
                us[s] = u
                nc.scalar.copy(u[:], av[:])

            # ---- epilogue for the last strip ----
            s = NSTRIP - 1
            fin_a(s)
            fin_b(s)
            fin_c(s)
            for m in range(4):
                for n in range(2):
                    outproj_chunk(s, m, n, cp_eng="s" if (2 * m + n) % 2 else "v")

    nc.compile()
    return nc


def kernel(q, k, v, w_q, b_q, w_k, b_k, w_v, b_v, w_o, b_o):
    global _CACHED_NC, LAST_RESULT
    if _CACHED_NC is None:
        _CACHED_NC = _build()
    nc = _CACHED_NC

    bf16 = ml_dtypes.bfloat16

    def tile_T(x):  # [L, D] -> [128, D//128, L] contiguous
        xt = np.asarray(x, np.float32)[0].T  # [D, L]
        return np.ascontiguousarray(
            xt.reshape(D // P, P, L).transpose(1, 0, 2)
        ).astype(bf16)

    def tile_w(w):  # [D, 128] -> [128, D//128, 128] contiguous
        return np.ascontiguousarray(
            w.reshape(D // P, P, P).transpose(1, 0, 2)
        ).astype(bf16)

    k2 = tile_T(k)
    # q strip-major: [NSTRIP, 128, KT, 512]
    q2 = np.ascontiguousarray(
        tile_T(q).reshape(P, KT, NSTRIP, SW).transpose(2, 0, 1, 3)
    )
    # v kt-major: [LT, 128, KT, 128]
    v2 = np.ascontiguousarray(
        tile_T(v).reshape(P, KT, LT, P).transpose(2, 0, 1, 3)
    )
    w_q = np.asarray(w_q, np.float32)
    w_k = np.asarray(w_k, np.float32)
    w_v = np.asarray(w_v, np.float32)
    w_o = np.asarray(w_o, np.float32)
    b_q = np.asarray(b_q, np.float32)
    b_v = np.asarray(b_v, np.float32)
    b_o = np.asarray(b_o, np.float32)

    in_maps = []
    for i in range(NCORES):
        sl = slice(P * i, P * (i + 1))
        in_maps.append(
            {
                "kT": k2,
                "qS": q2,
                "vK": v2,
                "wq": tile_w(w_q[:, sl]),
                "wk": tile_w(w_k[:, sl]),
                "wv": tile_w(w_v[:, sl]),
                "bq": np.ascontiguousarray(b_q[sl]).reshape(P, 1),
                "wo": np.ascontiguousarray(w_o[sl, :]).astype(bf16),
            }
        )

    kwargs = {}
    if TRACE:
        import shutil

        tdir = "/tmp/bass_trace"
        shutil.rmtree(tdir, ignore_errors=True)
        os.makedirs(tdir, exist_ok=True)
        kwargs["tmpdir"] = tdir
    res = run_bass_kernel_spmd(nc, in_maps, list(range(NCORES)), trace=TRACE, **kwargs)
    LAST_RESULT = {
        "exec_time_ns": res.exec_time_ns,
        "trace_path": (res.instructions_and_trace or (None, None))[1],
    }
    acc = np.zeros((L, D), np.float64)
    for i in range(NCORES):
        acc += res.results[i]["out"].astype(np.float64)
    # b_k cancels in softmax; b_v and b_o contribute a constant output row
    acc += (b_o + b_v @ w_o).astype(np.float64)
    return acc.astype(np.float32).reshape(1, L, D)


# revision 13
# speedup vs baseline: 1.1106x; 1.0440x over previous
"""Multi-head attention (B=1, L=2048, D=1024, H=16) on 8 TRN2 NeuronCores.

Sharding: tensor-parallel over heads. Core i computes heads 2i, 2i+1:
  - projections with column shards of w_q/w_k/w_v (128 cols each)
  - full attention for its 2 heads
  - partial output projection with the matching 128-row shard of w_o
Host sums the 8 partial outputs and adds the fused bias b_o + b_v @ w_o
(b_v contributes a constant row to the output; b_k cancels in softmax).

Strip-pipelined schedule (all matmuls bf16, fp32 PSUM):
  - q processed in 4 strips of 512; per (strip, kt) iteration:
      S^T pair (row-tiled K=64 matmuls, heads at PE row groups 0/64)
      -> ONE exp over [128, 1024] (both heads, single PSUM tile)
      -> AV pair (col-tiled M=64, heads at PSUM partition groups 0/64)
      -> denominator accumulate split across VectorE / GpSimd by kt parity
  - tensor queue padded with out-projection chunks of strip s-1,
    q-projection of strip s+1, vh blocks, and normalize matmuls so the
    PE never idles (keeps the 2.4 GHz p-state)
  - host supplies q strip-major and v kt-major so DMA descriptors stay
    large and vh blocks become available incrementally
  - per-strip denominator reciprocal via partition-spread DMA
"""

import os
import numpy as np
import ml_dtypes

import concourse.bass as bass
import concourse.mybir as mybir
import concourse.tile as tile
from concourse import bacc
from concourse.bass import ts
from concourse.bass_utils import run_bass_kernel_spmd

P = 128
L = 2048
D = 1024
DH = 64
NCORES = 8
NSTRIP = 4
SW = 512  # strip width (q columns per strip)
KT = D // P  # 8 contraction tiles for the projections
LT = L // P  # 16 seq tiles
BF16 = mybir.dt.bfloat16
F32 = mybir.dt.float32
AF = mybir.ActivationFunctionType
ALU = mybir.AluOpType

TRACE = False  # test.py flips this to get an NTFF profile / exec_time_ns
LAST_RESULT = {}

_CACHED_NC = None


def _build():
    nc = bacc.Bacc("TRN2", target_bir_lowering=False, debug=False, num_devices=NCORES)

    kS = nc.dram_tensor("kS", [NSTRIP, P, KT, SW], BF16, kind="ExternalInput")
    qS = nc.dram_tensor("qS", [NSTRIP, P, KT, SW], BF16, kind="ExternalInput")
    vK = nc.dram_tensor("vK", [LT, P, KT, P], BF16, kind="ExternalInput")
    wq = nc.dram_tensor("wq", [P, KT, P], BF16, kind="ExternalInput")
    wk = nc.dram_tensor("wk", [P, KT, P], BF16, kind="ExternalInput")
    wv = nc.dram_tensor("wv", [P, KT, P], BF16, kind="ExternalInput")
    bq = nc.dram_tensor("bq", [P, 1], F32, kind="ExternalInput")
    wo = nc.dram_tensor("wo", [P, D], BF16, kind="ExternalInput")
    out = nc.dram_tensor("out", [L, D], BF16, kind="ExternalOutput")

    with tile.TileContext(nc) as tc:
        with (
            tc.tile_pool(name="const", bufs=1) as const_pool,
            tc.tile_pool(name="inputs", bufs=1) as in_pool,
            tc.tile_pool(name="proj", bufs=1) as proj_pool,
            tc.tile_pool(name="work", bufs=1) as work_pool,
            tc.tile_pool(name="ps", bufs=1, space="PSUM") as psp,
            tc.tile_pool(name="ptp", bufs=3) as pt_pool,
            tc.tile_pool(name="accp", bufs=2) as acc_pool,
            tc.tile_pool(name="up", bufs=2) as u_pool,
            tc.tile_pool(name="osbp", bufs=2) as osb_pool,
        ):
            ones_c = const_pool.tile([P, P], BF16)
            nc.vector.memset(ones_c[:], 1.0)
            scr = const_pool.tile([1, 32], F32)
            nc.scalar.activation(scr[:], ones_c[0:1, 0:32], AF.Exp)

            # ---- input DMAs: weights, then k, then q strip0, v blocks,
            # then remaining q strips ----
            wq_sb = in_pool.tile([P, KT, P], BF16)
            wk_sb = in_pool.tile([P, KT, P], BF16)
            wv_sb = in_pool.tile([P, KT, P], BF16)
            bq_sb = in_pool.tile([P, 1], F32)
            wo_sb = in_pool.tile([P, D], BF16)
            nc.sync.dma_start(wk_sb[:], wk[:])
            nc.scalar.dma_start(wq_sb[:], wq[:])
            nc.gpsimd.dma_start(wv_sb[:], wv[:])
            nc.scalar.dma_start(bq_sb[:], bq[:])
            nc.gpsimd.dma_start(wo_sb[:], wo[:])

            kS_sb = in_pool.tile([P, NSTRIP, KT, SW], BF16)
            qS_sb = in_pool.tile([P, NSTRIP, KT, SW], BF16)
            vK_sb = in_pool.tile([P, LT, KT, P], BF16)
            # minimal upfront preamble: kS0/kS1, qS0, vK0-4; everything
            # else is released later, paced by the scalar queue's exps
            nc.sync.dma_start(kS_sb[:, 0, :, :], kS[0])
            nc.scalar.dma_start(qS_sb[:, 0, :, :], qS[0])
            nc.gpsimd.dma_start(kS_sb[:, 1, :, :], kS[1])
            nc.sync.dma_start(vK_sb[:, 0, :, :], vK[0])
            nc.gpsimd.dma_start(vK_sb[:, 1, :, :], vK[1])
            nc.sync.dma_start(vK_sb[:, 2, :, :], vK[2])
            nc.gpsimd.dma_start(vK_sb[:, 3, :, :], vK[3])
            nc.sync.dma_start(vK_sb[:, 4, :, :], vK[4])

            def paced_dma(s, kt):
                """Deferred input DMAs, triggered on the scalar queue right
                after exp(s, kt) so the stream is paced by loop progress."""
                if s == 0:
                    if kt == 0:
                        nc.scalar.dma_start(kS_sb[:, 2, :, :], kS[2])
                    elif kt == 1:
                        nc.scalar.dma_start(kS_sb[:, 3, :, :], kS[3])
                        nc.scalar.dma_start(vK_sb[:, 5, :, :], vK[5])
                    elif kt == 2:
                        nc.scalar.dma_start(qS_sb[:, 1, :, :], qS[1])
                        nc.scalar.dma_start(vK_sb[:, 6, :, :], vK[6])
                    elif 3 <= kt <= 11:
                        nc.scalar.dma_start(vK_sb[:, kt + 4, :, :], vK[kt + 4])
                elif s < NSTRIP - 1 and kt == 1:
                    nc.scalar.dma_start(qS_sb[:, s + 1, :, :], qS[s + 1])

            # ---- persistent SBUF tensors ----
            khT = proj_pool.tile([P, L], BF16)
            qhT = proj_pool.tile([P, L], BF16)
            vh_sb = proj_pool.tile([P, LT, P], BF16)  # [kseq, kt, dh-pair]
            lhsT_c = work_pool.tile([P, L], BF16)  # normalized concat^T
            rdb = work_pool.tile([1, NSTRIP, 2 * SW], BF16)  # 1/denominators

            def proj_k_chunk(n):
                """khT[:, n*512:(n+1)*512] (no bias: b_k cancels in softmax)."""
                ps = psp.tile([P, SW], F32, tag="mm", bufs=2, name=f"mmk_{n}")
                for t in range(KT):
                    nc.tensor.matmul(
                        ps[:],
                        wk_sb[:, t, :],
                        kS_sb[:, n, t, :],
                        start=(t == 0),
                        stop=(t == KT - 1),
                    )
                nc.scalar.copy(khT[:, ts(n, SW)], ps[:])

            _qps = {}

            def proj_q_part(s, half):
                """Half of the q-projection for strip s (pad-slot sized)."""
                if half == 0:
                    _qps[s] = psp.tile(
                        [P, SW], F32, tag="mm", bufs=2, name=f"mmq_{s}"
                    )
                ps = _qps[s]
                for t in range(4 * half, 4 * half + 4):
                    nc.tensor.matmul(
                        ps[:],
                        wq_sb[:, t, :],
                        qS_sb[:, s, t, :],
                        start=(t == 0),
                        stop=(t == KT - 1),
                    )
                if half == 1:
                    nc.vector.tensor_scalar(
                        qhT[:, ts(s, SW)], ps[:], bq_sb[:], None, op0=ALU.add
                    )

            def proj_q_strip(s):
                proj_q_part(s, 0)
                proj_q_part(s, 1)

            def vh_block(b):
                """vh_sb[:, b, :] = (v @ w_v)[b-th kseq tile] directly."""
                ps = psp.tile([P, SW], F32, tag="mm", bufs=2, name=f"mmv_{b}")
                for t in range(KT):
                    nc.tensor.matmul(
                        ps[:, 0:P],
                        vK_sb[:, b, t, :],
                        wv_sb[:, t, :],
                        start=(t == 0),
                        stop=(t == KT - 1),
                    )
                nc.vector.tensor_copy(vh_sb[:, b, :], ps[:, 0:P])

            # ---- prologue ----
            proj_k_chunk(0)
            proj_q_strip(0)

            # ---- strip-pipelined attention ----
            accs = [None] * NSTRIP
            us = [None] * NSTRIP
            dsps = [None] * NSTRIP
            osbs = {}

            def fin_a(s):
                """Denominator column-sums of strip s + spread DMA."""
                acc_g, acc_v = accs[s]
                nc.vector.tensor_tensor(acc_v[:], acc_v[:], acc_g[:], op=ALU.add)
                dps = psp.tile([P, SW], F32, tag="mm", bufs=2, name=f"dcs_{s}")
                for h in (0, 1):
                    nc.tensor.matmul(
                        dps[32 * h : 32 * h + 1, :],
                        ones_c[:, 0:1],
                        acc_v[:, ts(h, SW)],
                    )
                dsb = work_pool.tile(
                    [1, 2 * SW], F32, tag="dsb", bufs=2, name=f"dsb_{s}"
                )
                nc.scalar.copy(dsb[0:1, 0:SW], dps[0:1, :])
                nc.scalar.copy(dsb[0:1, SW : 2 * SW], dps[32:33, :])
                dsp = work_pool.tile([P, 8], F32, tag="dsp", bufs=2, name=f"dsp_{s}")
                dsps[s] = dsp
                nc.sync.dma_start(dsp[0:DH, :], dsb[0:1, 0:SW])
                nc.gpsimd.dma_start(dsp[DH:P, :], dsb[0:1, SW : 2 * SW])

            def fin_b(s):
                """Reciprocal on the spread layout + gather back."""
                dsp = dsps[s]
                nc.vector.reciprocal(dsp[:], dsp[:])
                dspb = work_pool.tile([P, 8], BF16, tag="dspb", bufs=2, name=f"dspb_{s}")
                nc.vector.tensor_copy(dspb[:], dsp[:])
                nc.sync.dma_start(rdb[0:1, s, 0:SW], dspb[0:DH, :])
                nc.gpsimd.dma_start(rdb[0:1, s, SW : 2 * SW], dspb[DH:P, :])

            def fin_c(s):
                """Broadcast 1/d over partitions, normalize -> lhsT_c."""
                bc = psp.tile([P, SW], F32, tag="mm", bufs=2, name=f"bc_{s}")
                for h in (0, 1):
                    nc.tensor.matmul(
                        bc[ts(h, DH), :],
                        ones_c[0:1, 0:DH],
                        rdb[0:1, s, ts(h, SW)],
                    )
                nc.vector.tensor_tensor(
                    lhsT_c[:, ts(s, SW)], us[s][:], bc[:], op=ALU.mult
                )

            def outproj_chunk(s, m, n, cp_eng="v"):
                """out[s*512 + m*128, n*512] partial chunk of strip s."""
                ps = psp.tile([P, SW], F32, tag="mm", bufs=2, name=f"op_{s}_{m}_{n}")
                nc.tensor.matmul(
                    ps[:], lhsT_c[:, ts(4 * s + m, P)], wo_sb[:, ts(n, SW)]
                )
                osb = osbs.get((s, m))
                if osb is None:
                    osb = osb_pool.tile([P, D], BF16, tag="osb", name=f"osb_{s}_{m}")
                    osbs[(s, m)] = osb
                (nc.scalar.copy if cp_eng == "s" else nc.vector.tensor_copy)(
                    osb[:, ts(n, SW)], ps[:]
                )
                if n == 1:
                    (nc.sync if m % 2 == 0 else nc.gpsimd).dma_start(
                        out[ts(4 * s + m, P), :], osb[:]
                    )

            for s in range(NSTRIP):
                av = psp.tile([P, SW], F32, tag="av", bufs=2, name=f"av_{s}")
                acc_g = acc_pool.tile([P, 2 * SW], BF16, tag="accg", name=f"accg_{s}")
                acc_v = acc_pool.tile([P, 2 * SW], BF16, tag="accv", name=f"accv_{s}")
                accs[s] = (acc_g, acc_v)
                pts = [None, None, None]
                for kt in range(LT):
                    # scores: both heads into one PSUM tile (row groups 0/64)
                    st = psp.tile(
                        [P, 2 * SW], F32, tag="st", bufs=2, name=f"st_{s}_{kt}"
                    )
                    for h in (0, 1):
                        nc.tensor.matmul(
                            st[:, ts(h, SW)],
                            khT[ts(h, DH), ts(kt, P)],
                            qhT[ts(h, DH), ts(s, SW)],
                        )
                    # one exp for both heads; scale 1/sqrt(64) folded in
                    pt = pt_pool.tile([P, 2 * SW], BF16, tag="pt", name=f"pt_{s}_{kt}")
                    pts[kt % 3] = pt
                    nc.scalar.activation(pt[:], st[:], AF.Exp, scale=0.125)
                    paced_dma(s, kt)

                    # tensor pad work (keeps PE busy while exp(kt) finishes)
                    if s == 0:
                        if kt == 0:
                            vh_block(0)
                            vh_block(1)
                            vh_block(2)
                        elif kt <= 13:
                            vh_block(kt + 2)
                        elif kt == 14:
                            proj_q_part(1, 0)
                        elif kt == 15:
                            proj_q_part(1, 1)
                        if kt == 1:
                            proj_k_chunk(1)
                        elif kt == 5:
                            proj_k_chunk(2)
                        elif kt == 9:
                            proj_k_chunk(3)
                    else:
                        if kt == 0:
                            fin_a(s - 1)
                        elif kt == 3:
                            fin_b(s - 1)
                        elif kt == 5:
                            fin_c(s - 1)
                        elif 6 <= kt <= 13:
                            outproj_chunk(s - 1, (kt - 6) // 2, (kt - 6) % 2)
                        elif kt == 14 and s + 1 < NSTRIP:
                            proj_q_part(s + 1, 0)
                        elif kt == 15 and s + 1 < NSTRIP:
                            proj_q_part(s + 1, 1)

                    # AV pair of the previous kt (exp long since done)
                    def av_pair(k):
                        for h in (0, 1):
                            nc.tensor.matmul(
                                av[ts(h, DH), :],
                                vh_sb[:, k, ts(h, DH)],
                                pts[k % 3][:, ts(h, SW)],
                                start=(k == 0),
                                stop=(k == LT - 1),
                            )

                    if kt > 0:
                        av_pair(kt - 1)
                    # two independent denominator chains; gpsimd (slow per
                    # op) only gets mid-strip slots so it never lags fin_a
                    if kt in (2, 5, 8, 11):
                        if kt == 2:
                            nc.gpsimd.tensor_copy(acc_g[:], pt[:])
                        else:
                            nc.gpsimd.tensor_tensor(
                                acc_g[:], acc_g[:], pt[:], op=ALU.add
                            )
                    else:
                        if kt == 0:
                            nc.vector.tensor_copy(acc_v[:], pt[:])
                        else:
                            nc.vector.tensor_tensor(
                                acc_v[:], acc_v[:], pt[:], op=ALU.add
                            )
                av_pair(LT - 1)
                # unnormalized attention out of this strip -> SBUF
                u = u_pool.tile([P, SW], F32, tag="u", name=f"u_{s}")
                us[s] = u
                nc.scalar.copy(u[:], av[:])

            # ---- epilogue for the last strip ----
            s = NSTRIP - 1
            fin_a(s)
            fin_b(s)
            fin_c(s)
            for m in range(4):
                for n in range(2):
                    outproj_chunk(s, m, n, cp_eng="s" if (2 * m + n) % 2 else "v")

    nc.compile()
    return nc


def kernel(q, k, v, w_q, b_q, w_k, b_k, w_v, b_v, w_o, b_o):
    global _CACHED_NC, LAST_RESULT
    if _CACHED_NC is None:
        _CACHED_NC = _build()
    nc = _CACHED_NC

    bf16 = ml_dtypes.bfloat16

    def tile_T(x):  # [L, D] -> [128, D//128, L] contiguous
        xt = np.asarray(x, np.float32)[0].T  # [D, L]
        return np.ascontiguousarray(
            xt.reshape(D // P, P, L).transpose(1, 0, 2)
        ).astype(bf16)

    def tile_w(w):  # [D, 128] -> [128, D//128, 128] contiguous
        return np.ascontiguousarray(
            w.reshape(D // P, P, P).transpose(1, 0, 2)
        ).astype(bf16)

    # k and q strip-major: [NSTRIP, 128, KT, 512]
    k2 = np.ascontiguousarray(
        tile_T(k).reshape(P, KT, NSTRIP, SW).transpose(2, 0, 1, 3)
    )
    q2 = np.ascontiguousarray(
        tile_T(q).reshape(P, KT, NSTRIP, SW).transpose(2, 0, 1, 3)
    )
    # v kt-major: [LT, 128, KT, 128]
    v2 = np.ascontiguousarray(
        tile_T(v).reshape(P, KT, LT, P).transpose(2, 0, 1, 3)
    )
    w_q = np.asarray(w_q, np.float32)
    w_k = np.asarray(w_k, np.float32)
    w_v = np.asarray(w_v, np.float32)
    w_o = np.asarray(w_o, np.float32)
    b_q = np.asarray(b_q, np.float32)
    b_v = np.asarray(b_v, np.float32)
    b_o = np.asarray(b_o, np.float32)

    in_maps = []
    for i in range(NCORES):
        sl = slice(P * i, P * (i + 1))
        in_maps.append(
            {
                "kS": k2,
                "qS": q2,
                "vK": v2,
                "wq": tile_w(w_q[:, sl]),
                "wk": tile_w(w_k[:, sl]),
                "wv": tile_w(w_v[:, sl]),
                "bq": np.ascontiguousarray(b_q[sl]).reshape(P, 1),
                "wo": np.ascontiguousarray(w_o[sl, :]).astype(bf16),
            }
        )

    kwargs = {}
    if TRACE:
        import shutil

        tdir = "/tmp/bass_trace"
        shutil.rmtree(tdir, ignore_errors=True)
        os.makedirs(tdir, exist_ok=True)
        kwargs["tmpdir"] = tdir
    res = run_bass_kernel_spmd(nc, in_maps, list(range(NCORES)), trace=TRACE, **kwargs)
    LAST_RESULT = {
        "exec_time_ns": res.exec_time_ns,
        "trace_path": (res.instructions_and_trace or (None, None))[1],
    }
    acc = np.zeros((L, D), np.float64)
    for i in range(NCORES):
        acc += res.results[i]["out"].astype(np.float64)
    # b_k cancels in softmax; b_v and b_o contribute a constant output row
    acc += (b_o + b_v @ w_o).astype(np.float64)
    return acc.astype(np.float32).reshape(1, L, D)
